# revision 4
# baseline (speedup 1.0000x reference)
"""MDN-RNN (LSTM + MDN heads) Trainium2 Bass kernel.

Sharding: data-parallel over batch B=64 -> 8 cores x 8 batch elements.
Per core:
  Phase 1: xg = W_ih @ x + b_ih + b_hh for all (t, b), written to DRAM scratch
           (bf16), computed as 8 gate-tile matmuls per 512-column chunk.
  Phase 2: recurrence over S=2048 steps in chunks of 64 steps:
           gates-on-partitions layout: PSUM G (128, 8 gate-tiles, 8 batch).
           Per step: identity-matmul injects xg chunk slice into PSUM
           (start=True), 16 accumulating matmuls add W_hh @ h_{t-1},
           ACT sigmoid/tanh -> DVE cell update -> h_t (bf16) written into the
           chunk's hs buffer (consumed directly as next step's moving operand).
           Per chunk: MDN head matmuls with hs as the stationary operand so the
           output lands (t,b)-major in PSUM -> softmax/exp postproc -> DMA out;
           hs transposed to (t,b)-major via PE transpose for the LSTM output.
"""

import sys

sys.path.insert(0, "/opt/trn_rl_repo")

import numpy as np
import ml_dtypes

import concourse.bass as bass
import concourse.tile as tile
from concourse import mybir
from concourse.bass_utils import run_bass_kernel_spmd

S, B, Z, A, H, M = 2048, 64, 32, 3, 256, 5
IN = Z + A  # 35
G4 = 4 * H  # 1024
MZ = M * Z  # 160
HD = 3 * MZ  # 480  (pi | sigma | mu)
TEMP = 1.3
NCORES = 8
BL = B // NCORES  # 8 batch elements per core
NT = S * BL  # 16384 (t, b) rows per core
CH = 512  # chunk width in (t, b) columns
TSTEPS = CH // BL  # 64 steps per chunk
NCHUNK = NT // CH  # 32

F32 = mybir.dt.float32
BF16 = mybir.dt.bfloat16
AF = mybir.ActivationFunctionType
OP = mybir.AluOpType
bf = ml_dtypes.bfloat16


def _trace(tc, nc, d):
    singles_cm = tc.tile_pool(name="singles", bufs=1)
    singles = singles_cm.__enter__()

    # --- persistent weights / constants -----------------------------------
    wih_sb = singles.tile([IN, G4], BF16)
    nc.sync.dma_start(out=wih_sb, in_=d["wihT"][:, :])
    bg_sb = singles.tile([128, 8], F32)
    nc.sync.dma_start(out=bg_sb, in_=d["bg"][:, :])
    whh_sb = singles.tile([128, 2, 8, 128], BF16)
    nc.sync.dma_start(
        out=whh_sb,
        in_=d["whhT"][:, :].rearrange("(k p) (g q) -> p k g q", p=128, q=128),
    )
    wh_sb = singles.tile([128, 2, HD], BF16)
    nc.sync.dma_start(out=wh_sb, in_=d["whT"][:, :].rearrange("(k p) n -> p k n", p=128))
    bh_sb = singles.tile([1, HD], BF16)
    nc.sync.dma_start(out=bh_sb, in_=d["bh"][:, :])
    id_sb = singles.tile([128, 128], BF16)
    nc.sync.dma_start(out=id_sb, in_=d["id128"][:, :])
    ones_sb = singles.tile([1, 128], BF16)
    nc.vector.memset(ones_sb, 1.0)

    # recurrent state
    c_sb = singles.tile([128, 2 * BL], F32)  # cell state, [k-tile, b] flat
    nc.vector.memset(c_sb, 0.0)
    hsch = singles.tile([128, 2, TSTEPS, BL], BF16)  # h per chunk (persistent)
    nc.vector.memset(hsch, 0.0)

    # DRAM scratch for xg (tracked via DRAM tile pool)
    dram_cm = tc.tile_pool(name="dramxg", bufs=1, space="DRAM")
    drampool = dram_cm.__enter__()
    xg_dr = drampool.tile([8, 128, NT], BF16)

    # --- Phase 1: xg precompute ------------------------------------------
    with (
        tc.tile_pool(name="p1x", bufs=3) as p1x,
        tc.tile_pool(name="p1ps", bufs=4, space="PSUM") as p1ps,
        tc.tile_pool(name="p1o", bufs=6) as p1o,
    ):
        for ch in range(NCHUNK):
            xt_sb = p1x.tile([IN, CH], BF16)
            nc.sync.dma_start(out=xt_sb, in_=d["xt"][:, ch * CH : (ch + 1) * CH])
            for gt in range(8):
                ps = p1ps.tile([128, CH], F32)
                nc.tensor.matmul(
                    ps,
                    wih_sb[:, gt * 128 : (gt + 1) * 128],
                    xt_sb,
                    start=True,
                    stop=True,
                )
                xo = p1o.tile([128, CH], BF16)
                if gt % 2 == 0:
                    nc.scalar.activation(
                        xo, ps, AF.Identity, bias=bg_sb[:, gt : gt + 1], scale=1.0
                    )
                else:
                    nc.vector.tensor_scalar_add(xo, ps, bg_sb[:, gt : gt + 1])
                nc.sync.dma_start(
                    out=xg_dr[gt, :, ch * CH : (ch + 1) * CH], in_=xo
                )

    # --- Phase 2: recurrence + heads -------------------------------------
    with (
        tc.tile_pool(name="xgp", bufs=1) as xgp,
        tc.tile_pool(name="pg", bufs=3) as pg,
        tc.tile_pool(name="ph", bufs=2) as ph,
        tc.tile_pool(name="pho", bufs=2) as pho,
        tc.tile_pool(name="psG", bufs=2, space="PSUM") as psG,
        tc.tile_pool(name="psH", bufs=2, space="PSUM") as psH,
        tc.tile_pool(name="psT", bufs=2, space="PSUM") as psT,
    ):
        hints = (
            mybir.EngineType.PE,
            mybir.EngineType.Activation,
            mybir.EngineType.DVE,
        )
        with tc.For_i(0, NT, CH, hint_engines=hints) as iv:
            xg_sb = xgp.tile([128, 8, CH], BF16)
            nc.sync.dma_start(
                out=xg_sb,
                in_=xg_dr[:, :, bass.ds(iv, CH)].rearrange("g p n -> p g n"),
            )
            for j in range(TSTEPS):
                Gp = psG.tile([128, 8, BL], F32)
                # inject xg (+biases, folded in phase 1) into PSUM
                nc.tensor.matmul(
                    Gp,
                    id_sb,
                    xg_sb[:, :, j * BL : (j + 1) * BL],
                    start=True,
                    stop=False,
                )
                jp = (j - 1) % TSTEPS  # previous step's h slot
                for gt in range(8):
                    for k in range(2):
                        nc.tensor.matmul(
                            Gp[:, gt, :],
                            whh_sb[:, k, gt, :],
                            hsch[:, k, jp, :],
                            start=False,
                            stop=(gt == 7 and k == 1),
                        )
                Gf = Gp.rearrange("p a b -> p (a b)")  # (128, 64)
                sif = pg.tile([128, 4 * BL], F32)
                nc.scalar.activation(sif, Gf[:, 0 : 4 * BL], AF.Sigmoid)
                tg = pg.tile([128, 2 * BL], F32)
                nc.scalar.activation(tg, Gf[:, 4 * BL : 6 * BL], AF.Tanh)
                so = pg.tile([128, 2 * BL], F32)
                nc.scalar.activation(so, Gf[:, 6 * BL : 8 * BL], AF.Sigmoid)
                t1 = pg.tile([128, 2 * BL], F32)
                nc.vector.tensor_tensor(t1, sif[:, 0 : 2 * BL], tg, OP.mult)
                t2 = pg.tile([128, 2 * BL], F32)
                nc.vector.tensor_tensor(t2, sif[:, 2 * BL : 4 * BL], c_sb, OP.mult)
                nc.vector.tensor_tensor(c_sb, t1, t2, OP.add)
                tcn = pg.tile([128, 2 * BL], F32)
                nc.scalar.activation(tcn, c_sb, AF.Tanh)
                nc.vector.tensor_tensor(
                    hsch[:, :, j, :],
                    so.rearrange("p (k b) -> p k b", k=2),
                    tcn.rearrange("p (k b) -> p k b", k=2),
                    OP.mult,
                )

            # ---- heads + hs output for this chunk -----------------------
            for m in range(4):
                hp = psH.tile([128, HD], F32)
                nc.tensor.matmul(hp, ones_sb, bh_sb, start=True, stop=False)
                for k in range(2):
                    nc.tensor.matmul(
                        hp,
                        hsch[:, k, m * 16 : (m + 1) * 16, :],
                        wh_sb[:, k, :],
                        start=False,
                        stop=(k == 1),
                    )
                es = ph.tile([128, 2 * MZ], F32)
                nc.scalar.activation(es, hp[:, 0 : 2 * MZ], AF.Exp)
                muo = pho.tile([128, MZ], F32)
                nc.scalar.copy(muo, hp[:, 2 * MZ : 3 * MZ])
                ssum = ph.tile([128, Z], F32)
                nc.vector.tensor_reduce(
                    ssum,
                    es[:, 0:MZ].rearrange("p (m z) -> p z m", m=M),
                    axis=mybir.AxisListType.X,
                    op=OP.add,
                )
                s2 = ph.tile([128, Z], F32)
                nc.vector.tensor_scalar_mul(s2, ssum, float(TEMP))
                rcp = ph.tile([128, Z], F32)
                nc.vector.reciprocal(rcp, s2)
                rcp_b = bass.AP(
                    tensor=rcp.tensor,
                    offset=rcp.offset,
                    ap=[rcp.ap[0], [0, M], rcp.ap[1]],
                )
                pio = pho.tile([128, MZ], F32)
                nc.vector.tensor_tensor(
                    pio.rearrange("p (m z) -> p m z", m=M),
                    es[:, 0:MZ].rearrange("p (m z) -> p m z", m=M),
                    rcp_b,
                    OP.mult,
                )
                rows = bass.ds(iv + m * 128, 128)
                nc.sync.dma_start(out=d["pi"][rows, :], in_=pio)
                nc.sync.dma_start(out=d["sg"][rows, :], in_=es[:, MZ : 2 * MZ])
                nc.sync.dma_start(out=d["mu"][rows, :], in_=muo)
                hso = pho.tile([128, H], F32)
                for k in range(2):
                    tp = psT.tile([128, 128], BF16)
                    nc.tensor.transpose(tp, hsch[:, k, m * 16 : (m + 1) * 16, :], id_sb)
                    nc.scalar.copy(hso[:, k * 128 : (k + 1) * 128], tp)
                nc.sync.dma_start(out=d["hs"][rows, :], in_=hso)

    drampool_exit = dram_cm.__exit__(None, None, None)
    singles_cm.__exit__(None, None, None)
    return drampool_exit


def _split_waits(nc, max_waits=1, max_updates=1):
    """Walrus in this container rejects instructions with more than ~1 sync
    wait; hoist extra waits onto same-engine EventSemaphore (wait-only)
    instructions placed immediately before, and spill extra updates onto
    update-only EventSemaphores immediately after (the trailing-nop pattern,
    safe per the PSUM doc: sequencer ops don't overtake engine completion
    semantics for updates emitted by Tile's clock)."""
    for f in nc.m.functions:
        for blk in f.blocks:
            out = []
            changed = False
            for inst in blk.instructions:
                si = inst.sync_info
                pre, post = [], []
                if si is not None and len(si.on_wait) > max_waits:
                    waits = list(si.on_wait)
                    extra, keep = waits[:-max_waits], waits[-max_waits:]
                    for w in extra:
                        pre.append(
                            mybir.InstEventSemaphore(
                                name=nc.get_next_instruction_name(),
                                engine=inst.engine,
                                ins=[],
                                outs=[],
                                sync_info=mybir.SyncInfo(on_wait=[w], on_update=[]),
                            )
                        )
                    si = mybir.SyncInfo(on_wait=keep, on_update=list(si.on_update))
                    inst.sync_info = si
                    changed = True
                if si is not None and len(si.on_update) > max_updates:
                    ups = list(si.on_update)
                    keep_u, extra_u = ups[:max_updates], ups[max_updates:]
                    for u in extra_u:
                        post.append(
                            mybir.InstEventSemaphore(
                                name=nc.get_next_instruction_name(),
                                engine=inst.engine,
                                ins=[],
                                outs=[],
                                sync_info=mybir.SyncInfo(on_wait=[], on_update=[u]),
                            )
                        )
                    inst.sync_info = mybir.SyncInfo(
                        on_wait=list(si.on_wait), on_update=keep_u
                    )
                    changed = True
                out.extend(pre)
                out.append(inst)
                out.extend(post)
            if changed:
                blk.instructions = out


def build():
    nc = bass.Bass("TRN2", target_bir_lowering=False, debug=False)
    d = {}
    d["xt"] = nc.declare_dram_parameter("xt", [IN, NT], BF16, isOutput=False)
    d["whhT"] = nc.declare_dram_parameter("whhT", [H, G4], BF16, isOutput=False)
    d["wihT"] = nc.declare_dram_parameter("wihT", [IN, G4], BF16, isOutput=False)
    d["bg"] = nc.declare_dram_parameter("bg", [128, 8], F32, isOutput=False)
    d["whT"] = nc.declare_dram_parameter("whT", [H, HD], BF16, isOutput=False)
    d["bh"] = nc.declare_dram_parameter("bh", [1, HD], BF16, isOutput=False)
    d["id128"] = nc.declare_dram_parameter("id128", [128, 128], BF16, isOutput=False)
    d["pi"] = nc.declare_dram_parameter("pi", [NT, MZ], F32, isOutput=True)
    d["sg"] = nc.declare_dram_parameter("sg", [NT, MZ], F32, isOutput=True)
    d["mu"] = nc.declare_dram_parameter("mu", [NT, MZ], F32, isOutput=True)
    d["hs"] = nc.declare_dram_parameter("hs", [NT, H], F32, isOutput=True)

    with tile.TileContext(nc) as tc:
        _trace(tc, nc, d)
    _split_waits(nc)
    return nc


def kernel(
    z,
    action,
    W_ih,
    W_hh,
    b_ih,
    b_hh,
    W_pi,
    b_pi,
    W_sigma,
    b_sigma,
    W_mu,
    b_mu,
    _trace_hw=False,
):
    z = np.asarray(z, np.float32)
    action = np.asarray(action, np.float32)
    W_ih = np.asarray(W_ih, np.float32)
    W_hh = np.asarray(W_hh, np.float32)
    b_ih = np.asarray(b_ih, np.float32)
    b_hh = np.asarray(b_hh, np.float32)

    x = np.concatenate([z, action], axis=-1)  # (S, B, 35)

    shared = {
        "whhT": W_hh.T.astype(bf),
        "wihT": W_ih.T.astype(bf),
        "bg": np.ascontiguousarray((b_ih + b_hh).reshape(8, 128).T, np.float32),
        "whT": np.concatenate(
            [np.asarray(W_pi), np.asarray(W_sigma), np.asarray(W_mu)], axis=0
        ).T.astype(bf),
        "bh": np.concatenate(
            [np.asarray(b_pi), np.asarray(b_sigma), np.asarray(b_mu)]
        ).reshape(1, HD).astype(bf),
        "id128": np.eye(128, dtype=np.float32).astype(bf),
    }
    in_maps = []
    for c in range(NCORES):
        xs = x[:, c * BL : (c + 1) * BL, :]  # (S, 8, 35)
        xt = np.ascontiguousarray(xs.transpose(2, 0, 1)).reshape(IN, NT).astype(bf)
        in_maps.append({"xt": xt, **shared})

    nc = build()
    res = run_bass_kernel_spmd(
        nc, in_maps, list(range(NCORES)), trace=_trace_hw
    )
    outs = res.results

    def gather(name, inner):
        parts = [
            np.asarray(outs[c][name], np.float32).reshape((S, BL) + inner)
            for c in range(NCORES)
        ]
        return np.concatenate(parts, axis=1)

    pi = gather("pi", (M, Z))
    sigma = gather("sg", (M, Z))
    mu = gather("mu", (M, Z))
    hs = gather("hs", (H,))
    kernel._last_exec_time_ns = getattr(res, "exec_time_ns", None)
    return pi, sigma, mu, hs
